# revision 4
# baseline (speedup 1.0000x reference)
"""Trainium2 Bass kernel for nn_EdgeDecoder (GNN edge decoder, 2 relations).

Strategy (data-parallel over edges, 8 NeuronCores):
  - Shard the 500k edges of each relation across 8 cores (62500/core).
  - Host resolves the edge->node addressing: per (core, relation) it lays
    out the fp16 embedding rows in edge order, pre-transposed to
    [dim, edge] streams, so the device reads them with plain bulk DMA
    (no SWDGE descriptors, no PE transposes). Same HBM bytes as a
    device-side gather; the Q7 descriptor-generation bottleneck is gone.
  - Per 4096-edge chunk, per 512-edge compute chunk (weight-stationary
    waves of 4, redundant LDWEIGHTS suppressed via ldweights=False):
      hT = relu(W1u^T guT + W1v^T gvT + b1)   (fp16 matmuls, f32 PSUM)
      logits = W2^T hT                        (fp16 matmul,  f32 PSUM)
    relu hc0 runs on Scalar (activation), relu hc1 on DVE (dual-op
    tensor_scalar); PSUM->SBUF logit copies alternate Scalar/DVE.
    b2 is added on host. Logits stay sharded; host reassembles.
"""
import sys

if "/opt/trn_rl_repo" not in sys.path:
    sys.path.insert(0, "/opt/trn_rl_repo")

import numpy as np

P = 128
D = 128
HID = 256
E = 500000
NCORES = 8
EPC = E // NCORES          # 62500 edges per core per relation
GCH = 4096                 # edges per stream chunk
CCH = 512                  # edges per compute chunk
WAVE = 6                   # compute chunks per weight-stationary wave
NREL = 2
NCHUNK = -(-EPC // GCH)    # 16 (last chunk ragged: 1060 edges)

_PROGRAM_CACHE = {}
LAST_RESULTS = None


def _build_program():
    import concourse.bacc as bacc
    import concourse.mybir as mybir
    from concourse.tile import TileContext

    f16, f32 = mybir.dt.float16, mybir.dt.float32

    nc = bacc.Bacc("TRN2", target_bir_lowering=False, debug=False)

    us_d, vs_d, outs = {}, {}, {}
    for r in range(NREL):
        us_d[r] = nc.dram_tensor(f"us{r}", [P, EPC], f16, kind="ExternalInput")
        vs_d[r] = nc.dram_tensor(f"vs{r}", [P, EPC], f16, kind="ExternalInput")
        outs[r] = nc.dram_tensor(f"o{r}", [1, EPC], f32, kind="ExternalOutput")
    w1u_d = [nc.dram_tensor(f"w1u{r}", [D, HID], f16, kind="ExternalInput")
             for r in range(NREL)]
    w1v_d = [nc.dram_tensor(f"w1v{r}", [D, HID], f16, kind="ExternalInput")
             for r in range(NREL)]
    w2_d = [nc.dram_tensor(f"w2{r}", [P, 2], f16, kind="ExternalInput")
            for r in range(NREL)]
    b1_d = [nc.dram_tensor(f"b1{r}", [P, 2], f32, kind="ExternalInput")
            for r in range(NREL)]

    def mm(out, lhsT, rhs, start, stop, first):
        return nc.tensor.matmul(out=out, lhsT=lhsT, rhs=rhs,
                                start=start, stop=stop)

    def _dedup_ldweights():
        ndrop = 0
        for blk in nc.main_func.blocks:
            last_key = None
            drops = []
            for inst in blk.instructions:
                if type(inst).__name__ == 'InstLdweights':
                    key = str(inst.ins[0])
                    si = inst.sync_info
                    busy = si is not None and (len(si.on_update) > 0
                                               or len(si.on_wait) > 0)
                    if key == last_key and not busy:
                        drops.append(inst)
                    else:
                        last_key = key
            for inst in drops:
                blk.instructions.remove(inst)
            ndrop += len(drops)
        return ndrop

    with TileContext(nc) as tc:
        with tc.tile_pool(name="sbw", bufs=1) as sbw, \
             tc.tile_pool(name="sbg", bufs=6) as sbg, \
             tc.tile_pool(name="sbh", bufs=12) as sbh, \
             tc.tile_pool(name="sblog", bufs=2) as sblog, \
             tc.tile_pool(name="ph", bufs=6, space="PSUM") as ph, \
             tc.tile_pool(name="pl", bufs=2, space="PSUM") as pl:

            w1u_t, w1v_t, w2_t, b1_t = [], [], [], []
            for r in range(NREL):
                t = sbw.tile([D, HID], f16, tag=f"w1u{r}")
                nc.sync.dma_start(out=t[:], in_=w1u_d[r].ap()[:])
                w1u_t.append(t)
                t = sbw.tile([D, HID], f16, tag=f"w1v{r}")
                nc.sync.dma_start(out=t[:], in_=w1v_d[r].ap()[:])
                w1v_t.append(t)
                t = sbw.tile([P, 2], f16, tag=f"w2{r}")
                nc.sync.dma_start(out=t[:], in_=w2_d[r].ap()[:])
                w2_t.append(t)
                t = sbw.tile([P, 2], f32, tag=f"b1{r}")
                nc.sync.dma_start(out=t[:], in_=b1_d[r].ap()[:])
                b1_t.append(t)

            for r in range(NREL):
                for c in range(NCHUNK):
                    clen = min(GCH, EPC - c * GCH)
                    ccw = [min(CCH, clen - i * CCH)
                           for i in range(-(-clen // CCH))]
                    gu = sbg.tile([P, GCH], f16, tag="gu")
                    nc.sync.dma_start(out=gu[:, :clen],
                                      in_=us_d[r].ap()[:, c * GCH:c * GCH + clen])
                    gv = sbg.tile([P, GCH], f16, tag="gv")
                    nc.sync.dma_start(out=gv[:, :clen],
                                      in_=vs_d[r].ap()[:, c * GCH:c * GCH + clen])

                    log_sb = sblog.tile([1, GCH], f32, tag="log")
                    for g in range(-(-len(ccw) // WAVE)):
                        js = [j for j in range(WAVE)
                              if g * WAVE + j < len(ccw)]
                        hts = [[None] * WAVE, [None] * WAVE]
                        for hc in range(2):
                            phw = {}
                            for j in js:
                                cc = g * WAVE + j
                                w = ccw[cc]
                                pht = ph.tile([P, CCH], f32, tag="ph")
                                mm(pht[:, :w],
                                   w1u_t[r][:, hc * P:(hc + 1) * P],
                                   gu[:, cc * CCH:cc * CCH + w],
                                   True, False, j == js[0])
                                phw[j] = pht
                            for j in js:
                                cc = g * WAVE + j
                                w = ccw[cc]
                                mm(phw[j][:, :w],
                                   w1v_t[r][:, hc * P:(hc + 1) * P],
                                   gv[:, cc * CCH:cc * CCH + w],
                                   False, True, j == js[0])
                            for j in js:
                                w = ccw[g * WAVE + j]
                                ht = sbh.tile([P, CCH], f16, tag="ht")
                                if (j + hc) % 2 == 0:
                                    nc.scalar.activation(
                                        out=ht[:, :w], in_=phw[j][:, :w],
                                        func=mybir.ActivationFunctionType.Relu,
                                        bias=b1_t[r][:, hc:hc + 1])
                                else:
                                    nc.vector.tensor_scalar(
                                        out=ht[:, :w], in0=phw[j][:, :w],
                                        scalar1=b1_t[r][:, hc:hc + 1],
                                        scalar2=0.0,
                                        op0=mybir.AluOpType.add,
                                        op1=mybir.AluOpType.max)
                                hts[hc][j] = ht
                        plts = {}
                        for j in js:
                            w = ccw[g * WAVE + j]
                            plt = pl.tile([1, CCH], f32, tag="pl")
                            mm(plt[:, :w], w2_t[r][:, 0:1],
                               hts[0][j][:, :w], True, False, j == js[0])
                            plts[j] = plt
                        for j in js:
                            w = ccw[g * WAVE + j]
                            mm(plts[j][:, :w], w2_t[r][:, 1:2],
                               hts[1][j][:, :w], False, True, j == js[0])
                        for j in js:
                            cc = g * WAVE + j
                            w = ccw[cc]
                            dst = log_sb[:, cc * CCH:cc * CCH + w]
                            if j % 2 == 0:
                                nc.scalar.activation(
                                    out=dst, in_=plts[j][:, :w],
                                    func=mybir.ActivationFunctionType.Copy)
                            else:
                                nc.vector.tensor_copy(out=dst,
                                                      in_=plts[j][:, :w])
                    nc.sync.dma_start(
                        out=outs[r].ap()[:, c * GCH:c * GCH + clen],
                        in_=log_sb[:, :clen])
    nd = _dedup_ldweights()
    import sys as _sys
    print(f"deduped {nd} ldweights", file=_sys.stderr)
    nc.compile()
    return nc


def _prep(user_embed, item_embed, u_clicks, v_clicks, u_buys, v_buys,
          W1_clicks, b1_clicks, W2_clicks, b2_clicks,
          W1_buys, b1_buys, W2_buys, b2_buys):
    user16 = np.asarray(user_embed, dtype=np.float32).astype(np.float16)
    item16 = np.asarray(item_embed, dtype=np.float32).astype(np.float16)
    rels = [
        (np.asarray(u_clicks), np.asarray(v_clicks),
         np.asarray(W1_clicks, np.float32), np.asarray(b1_clicks, np.float32),
         np.asarray(W2_clicks, np.float32), np.asarray(b2_clicks, np.float32)),
        (np.asarray(u_buys), np.asarray(v_buys),
         np.asarray(W1_buys, np.float32), np.asarray(b1_buys, np.float32),
         np.asarray(W2_buys, np.float32), np.asarray(b2_buys, np.float32)),
    ]

    in_maps = []
    b2s = []
    for k in range(NCORES):
        m = {}
        for r in range(NREL):
            u_all, v_all, W1, b1, W2, b2 = rels[r]
            m[f"w1u{r}"] = W1[:D].astype(np.float16)
            m[f"w1v{r}"] = W1[D:].astype(np.float16)
            m[f"w2{r}"] = W2.reshape(2, P).T.astype(np.float16).copy()
            m[f"b1{r}"] = b1.reshape(2, P).T.astype(np.float32).copy()
            if k == 0:
                b2s.append(float(b2[0]))
            lo = k * EPC
            ue = np.asarray(u_all[lo:lo + EPC], np.int64)
            ve = np.asarray(v_all[lo:lo + EPC], np.int64)
            m[f"us{r}"] = np.ascontiguousarray(user16[ue].T)   # [128, EPC]
            m[f"vs{r}"] = np.ascontiguousarray(item16[ve].T)
        in_maps.append(m)
    return in_maps, b2s


def kernel(**inputs):
    global LAST_RESULTS
    from concourse import bass_utils

    in_maps, b2s = _prep(**inputs)

    if "prog" not in _PROGRAM_CACHE:
        _PROGRAM_CACHE["prog"] = _build_program()
    nc = _PROGRAM_CACHE["prog"]

    res = bass_utils.run_bass_kernel_spmd(nc, in_maps, core_ids=list(range(NCORES)))
    LAST_RESULTS = res

    outs = []
    for r in range(NREL):
        full = np.empty(E, np.float32)
        for k in range(NCORES):
            o = res.results[k][f"o{r}"].reshape(EPC)
            full[k * EPC:(k + 1) * EPC] = o
        if b2s[r] != 0.0:
            full += b2s[r]
        outs.append(full)
    return outs[0], outs[1]


# revision 5
# speedup vs baseline: 1.1886x; 1.1886x over previous
"""Trainium2 Bass kernel for nn_EdgeDecoder (GNN edge decoder, 2 relations).

Strategy (data-parallel over edges, 8 NeuronCores):
  - Shard the 500k edges of each relation across 8 cores (62500/core).
  - Host resolves the edge->node addressing: per (core, relation) it lays
    out the fp16 embedding rows in edge order, pre-transposed to
    [dim, edge] streams, so the device reads them with plain bulk DMA
    (no SWDGE descriptors, no PE transposes). Same HBM bytes as a
    device-side gather; the Q7 descriptor-generation bottleneck is gone.
  - Per 4096-edge chunk, per 512-edge compute chunk (weight-stationary
    waves of 4, redundant LDWEIGHTS suppressed via ldweights=False):
      hT = relu(W1u^T guT + W1v^T gvT + b1)   (fp16 matmuls, f32 PSUM)
      logits = W2^T hT                        (fp16 matmul,  f32 PSUM)
    relu hc0 runs on Scalar (activation), relu hc1 on DVE (dual-op
    tensor_scalar); PSUM->SBUF logit copies alternate Scalar/DVE.
    b2 is added on host. Logits stay sharded; host reassembles.
"""
import sys

if "/opt/trn_rl_repo" not in sys.path:
    sys.path.insert(0, "/opt/trn_rl_repo")

import numpy as np

P = 128
D = 128
HID = 256
E = 500000
NCORES = 8
EPC = E // NCORES          # 62500 edges per core per relation
GCH = 4096                 # edges per stream chunk
CCH = 512                  # edges per compute chunk
WAVE = 4                   # compute chunks per weight-stationary wave
NREL = 2
NCHUNK = -(-EPC // GCH)    # 16 (last chunk ragged: 1060 edges)

_PROGRAM_CACHE = {}
LAST_RESULTS = None


def _build_program():
    import concourse.bacc as bacc
    import concourse.mybir as mybir
    from concourse.tile import TileContext

    f16, f32 = mybir.dt.float16, mybir.dt.float32

    nc = bacc.Bacc("TRN2", target_bir_lowering=False, debug=False)

    us_d, vs_d, outs = {}, {}, {}
    for r in range(NREL):
        us_d[r] = nc.dram_tensor(f"us{r}", [P, EPC], f16, kind="ExternalInput")
        vs_d[r] = nc.dram_tensor(f"vs{r}", [P, EPC], f16, kind="ExternalInput")
        outs[r] = nc.dram_tensor(f"o{r}", [1, EPC], f32, kind="ExternalOutput")
    w1u_d = [nc.dram_tensor(f"w1u{r}", [D, HID], f16, kind="ExternalInput")
             for r in range(NREL)]
    w1v_d = [nc.dram_tensor(f"w1v{r}", [D, HID], f16, kind="ExternalInput")
             for r in range(NREL)]
    w2_d = [nc.dram_tensor(f"w2{r}", [P, 2], f16, kind="ExternalInput")
            for r in range(NREL)]
    b1_d = [nc.dram_tensor(f"b1{r}", [P, 2], f32, kind="ExternalInput")
            for r in range(NREL)]

    def mm(out, lhsT, rhs, start, stop, first):
        return nc.tensor.matmul(out=out, lhsT=lhsT, rhs=rhs,
                                start=start, stop=stop)

    def _dedup_ldweights():
        ndrop = 0
        for blk in nc.main_func.blocks:
            last_key = None
            drops = []
            for inst in blk.instructions:
                if type(inst).__name__ == 'InstLdweights':
                    key = str(inst.ins[0])
                    si = inst.sync_info
                    busy = si is not None and (len(si.on_update) > 0
                                               or len(si.on_wait) > 0)
                    if key == last_key and not busy:
                        drops.append(inst)
                    else:
                        last_key = key
            for inst in drops:
                blk.instructions.remove(inst)
            ndrop += len(drops)
        return ndrop

    with TileContext(nc) as tc:
        with tc.tile_pool(name="sbw", bufs=1) as sbw, \
             tc.tile_pool(name="sbg", bufs=6) as sbg, \
             tc.tile_pool(name="sbh", bufs=2 * WAVE) as sbh, \
             tc.tile_pool(name="sblog", bufs=2) as sblog, \
             tc.tile_pool(name="ph", bufs=WAVE, space="PSUM") as ph, \
             tc.tile_pool(name="pl", bufs=WAVE, space="PSUM") as pl:

            w1u_t, w1v_t, w2_t, b1_t = [], [], [], []
            for r in range(NREL):
                t = sbw.tile([D, HID], f16, tag=f"w1u{r}")
                nc.sync.dma_start(out=t[:], in_=w1u_d[r].ap()[:])
                w1u_t.append(t)
                t = sbw.tile([D, HID], f16, tag=f"w1v{r}")
                nc.sync.dma_start(out=t[:], in_=w1v_d[r].ap()[:])
                w1v_t.append(t)
                t = sbw.tile([P, 2], f16, tag=f"w2{r}")
                nc.sync.dma_start(out=t[:], in_=w2_d[r].ap()[:])
                w2_t.append(t)
                t = sbw.tile([P, 2], f32, tag=f"b1{r}")
                nc.sync.dma_start(out=t[:], in_=b1_d[r].ap()[:])
                b1_t.append(t)

            for r in range(NREL):
                for c in range(NCHUNK):
                    clen = min(GCH, EPC - c * GCH)
                    ccw = [min(CCH, clen - i * CCH)
                           for i in range(-(-clen // CCH))]
                    gu = sbg.tile([P, GCH], f16, tag="gu")
                    nc.sync.dma_start(out=gu[:, :clen],
                                      in_=us_d[r].ap()[:, c * GCH:c * GCH + clen])
                    gv = sbg.tile([P, GCH], f16, tag="gv")
                    nc.sync.dma_start(out=gv[:, :clen],
                                      in_=vs_d[r].ap()[:, c * GCH:c * GCH + clen])

                    log_sb = sblog.tile([1, GCH], f32, tag="log")
                    for g in range(-(-len(ccw) // WAVE)):
                        js = [j for j in range(WAVE)
                              if g * WAVE + j < len(ccw)]
                        hts = [[None] * WAVE, [None] * WAVE]
                        for hc in range(2):
                            phw = {}
                            for j in js:
                                cc = g * WAVE + j
                                w = ccw[cc]
                                pht = ph.tile([P, CCH], f32, tag="ph")
                                mm(pht[:, :w],
                                   w1u_t[r][:, hc * P:(hc + 1) * P],
                                   gu[:, cc * CCH:cc * CCH + w],
                                   True, False, j == js[0])
                                phw[j] = pht
                            for j in js:
                                cc = g * WAVE + j
                                w = ccw[cc]
                                mm(phw[j][:, :w],
                                   w1v_t[r][:, hc * P:(hc + 1) * P],
                                   gv[:, cc * CCH:cc * CCH + w],
                                   False, True, j == js[0])
                            for j in js:
                                w = ccw[g * WAVE + j]
                                ht = sbh.tile([P, CCH], f16, tag="ht")
                                if (j + hc) % 2 == 0:
                                    nc.scalar.activation(
                                        out=ht[:, :w], in_=phw[j][:, :w],
                                        func=mybir.ActivationFunctionType.Relu,
                                        bias=b1_t[r][:, hc:hc + 1])
                                else:
                                    nc.vector.tensor_scalar(
                                        out=ht[:, :w], in0=phw[j][:, :w],
                                        scalar1=b1_t[r][:, hc:hc + 1],
                                        scalar2=0.0,
                                        op0=mybir.AluOpType.add,
                                        op1=mybir.AluOpType.max)
                                hts[hc][j] = ht
                        plts = {}
                        for j in js:
                            w = ccw[g * WAVE + j]
                            plt = pl.tile([1, CCH], f32, tag="pl")
                            mm(plt[:, :w], w2_t[r][:, 0:1],
                               hts[0][j][:, :w], True, False, j == js[0])
                            plts[j] = plt
                        for j in js:
                            w = ccw[g * WAVE + j]
                            mm(plts[j][:, :w], w2_t[r][:, 1:2],
                               hts[1][j][:, :w], False, True, j == js[0])
                        for j in js:
                            cc = g * WAVE + j
                            w = ccw[cc]
                            dst = log_sb[:, cc * CCH:cc * CCH + w]
                            if j % 2 == 0:
                                nc.scalar.activation(
                                    out=dst, in_=plts[j][:, :w],
                                    func=mybir.ActivationFunctionType.Copy)
                            else:
                                nc.vector.tensor_copy(out=dst,
                                                      in_=plts[j][:, :w])
                    nc.sync.dma_start(
                        out=outs[r].ap()[:, c * GCH:c * GCH + clen],
                        in_=log_sb[:, :clen])
    nd = _dedup_ldweights()
    import sys as _sys
    print(f"deduped {nd} ldweights", file=_sys.stderr)
    nc.compile()
    return nc


def _prep(user_embed, item_embed, u_clicks, v_clicks, u_buys, v_buys,
          W1_clicks, b1_clicks, W2_clicks, b2_clicks,
          W1_buys, b1_buys, W2_buys, b2_buys):
    user16 = np.asarray(user_embed, dtype=np.float32).astype(np.float16)
    item16 = np.asarray(item_embed, dtype=np.float32).astype(np.float16)
    rels = [
        (np.asarray(u_clicks), np.asarray(v_clicks),
         np.asarray(W1_clicks, np.float32), np.asarray(b1_clicks, np.float32),
         np.asarray(W2_clicks, np.float32), np.asarray(b2_clicks, np.float32)),
        (np.asarray(u_buys), np.asarray(v_buys),
         np.asarray(W1_buys, np.float32), np.asarray(b1_buys, np.float32),
         np.asarray(W2_buys, np.float32), np.asarray(b2_buys, np.float32)),
    ]

    in_maps = []
    b2s = []
    for k in range(NCORES):
        m = {}
        for r in range(NREL):
            u_all, v_all, W1, b1, W2, b2 = rels[r]
            m[f"w1u{r}"] = W1[:D].astype(np.float16)
            m[f"w1v{r}"] = W1[D:].astype(np.float16)
            m[f"w2{r}"] = W2.reshape(2, P).T.astype(np.float16).copy()
            m[f"b1{r}"] = b1.reshape(2, P).T.astype(np.float32).copy()
            if k == 0:
                b2s.append(float(b2[0]))
            lo = k * EPC
            ue = np.asarray(u_all[lo:lo + EPC], np.int64)
            ve = np.asarray(v_all[lo:lo + EPC], np.int64)
            m[f"us{r}"] = np.ascontiguousarray(user16[ue].T)   # [128, EPC]
            m[f"vs{r}"] = np.ascontiguousarray(item16[ve].T)
        in_maps.append(m)
    return in_maps, b2s


def kernel(**inputs):
    global LAST_RESULTS
    from concourse import bass_utils

    in_maps, b2s = _prep(**inputs)

    if "prog" not in _PROGRAM_CACHE:
        _PROGRAM_CACHE["prog"] = _build_program()
    nc = _PROGRAM_CACHE["prog"]

    res = bass_utils.run_bass_kernel_spmd(nc, in_maps, core_ids=list(range(NCORES)))
    LAST_RESULTS = res

    outs = []
    for r in range(NREL):
        full = np.empty(E, np.float32)
        for k in range(NCORES):
            o = res.results[k][f"o{r}"].reshape(EPC)
            full[k * EPC:(k + 1) * EPC] = o
        if b2s[r] != 0.0:
            full += b2s[r]
        outs.append(full)
    return outs[0], outs[1]
